# revision 25
# baseline (speedup 1.0000x reference)
import sys

if "/opt/trn_rl_repo" not in sys.path:
    sys.path.insert(0, "/opt/trn_rl_repo")

import numpy as np

B, S, D, NH, DH = 4, 2048, 768, 12, 64
NHL = 6        # heads per core
NPAIR = 3      # head pairs per core
NCH = 6        # d_model chunks of 128
NT = 16        # seq tiles of 128
NSQ = 4        # sq chunks of 512

_CACHE = {}


def build_nc(body_reps=1):
    import concourse.tile as tile
    from concourse import mybir, bacc

    f32 = mybir.dt.float32
    bf16 = mybir.dt.bfloat16
    AF = mybir.ActivationFunctionType

    nc = bacc.Bacc("TRN2", target_bir_lowering=False, debug=False)

    xT_d = nc.dram_tensor("xT", [2 * NCH * 128, S // 2], bf16, kind="ExternalInput")
    wq_d = nc.dram_tensor("wq", [128, NPAIR * NCH * 128], bf16, kind="ExternalInput")
    wk_d = nc.dram_tensor("wk", [128, NPAIR * NCH * 128], bf16, kind="ExternalInput")
    wv_d = nc.dram_tensor("wv", [128, NCH * 384], bf16, kind="ExternalInput")
    wo_d = nc.dram_tensor("wo", [128, NPAIR * 768], bf16, kind="ExternalInput")
    bqk_d = nc.dram_tensor("bqk", [128, 6], f32, kind="ExternalInput")
    bvb_d = nc.dram_tensor("bvb", [128, 384], f32, kind="ExternalInput")
    mask_d = nc.dram_tensor("maskT", [128, 128], bf16, kind="ExternalInput")
    eye_d = nc.dram_tensor("eye", [128, 128], bf16, kind="ExternalInput")
    out_d = nc.dram_tensor("out", [S, D], bf16, kind="ExternalOutput")

    with tile.TileContext(nc) as tc:
        for rep in range(body_reps):
            _emit_body(nc, tc, tile, mybir, rep,
                       xT_d, wq_d, wk_d, wv_d, wo_d, bqk_d, bvb_d, mask_d, eye_d, out_d)

    nc.compile()
    return nc


def _emit_body(nc, tc, tile, mybir, rep,
               xT_d, wq_d, wk_d, wv_d, wo_d, bqk_d, bvb_d, mask_d, eye_d, out_d):
    f32 = mybir.dt.float32
    bf16 = mybir.dt.bfloat16
    AF = mybir.ActivationFunctionType
    R = f"r{rep}"

    with (
        tc.tile_pool(name=f"sb{rep}", bufs=1) as sb,
        tc.tile_pool(name=f"psum{rep}", bufs=1, space="PSUM") as psum,
    ):
        # ---- constants ----
        wo_sb = sb.tile([128, NPAIR, 768], bf16, tag="wo")
        bqk_sb = sb.tile([128, 6], f32, tag="bqk")
        bvb_sb = sb.tile([128, 384], f32, tag="bvb")
        mask_sb = sb.tile([128, 128], bf16, tag="mask")
        eye_sb = sb.tile([128, 128], bf16, tag="eye")
        one_sb = sb.tile([1, 1], f32, tag="one1")
        xT_sb = sb.tile([128, NCH, S], bf16, tag="xT")
        wq_sb = sb.tile([128, NPAIR, NCH, 128], bf16, tag="wq")
        wk_sb = sb.tile([128, NPAIR, NCH, 128], bf16, tag="wk")
        wv_sb = sb.tile([128, NCH, 384], bf16, tag="wv")

        # xT streams on the SP queue (hf-outer so the first 1024 seq positions
        # of every d_model chunk land first); weights go down the Activation
        # hwdge queue concurrently so v/qk projections can start early.
        for hf in range(2):
            for c in range(NCH):
                pc = (hf * NCH + c) * 128
                nc.sync.dma_start(
                    xT_sb[:, c, hf * (S // 2):(hf + 1) * (S // 2)],
                    xT_d[pc:pc + 128, :],
                )
        nc.scalar.dma_start(wv_sb[:], wv_d[:].rearrange("k (c f) -> k c f", c=NCH))
        nc.scalar.dma_start(bvb_sb[:], bvb_d[:])
        nc.scalar.dma_start(wq_sb[:], wq_d[:].rearrange("k (p c m) -> k p c m", p=NPAIR, c=NCH))
        nc.scalar.dma_start(wk_sb[:], wk_d[:].rearrange("k (p c m) -> k p c m", p=NPAIR, c=NCH))
        nc.scalar.dma_start(bqk_sb[:], bqk_d[:])
        nc.scalar.dma_start(mask_sb[:], mask_d[:])
        nc.scalar.dma_start(eye_sb[:], eye_d[:])
        nc.scalar.dma_start(wo_sb[:], wo_d[:].rearrange("k (p d) -> k p d", p=NPAIR))
        nc.vector.memset(one_sb[:], 1.0)

        qT_sb = [sb.tile([128, S], bf16, tag=f"qT{p}", name=f"qT{p}{R}") for p in range(NPAIR)]
        kT_sb = [sb.tile([128, S], bf16, tag=f"kT{p}", name=f"kT{p}{R}") for p in range(NPAIR)]
        v_sb = [sb.tile([128, NHL, 65], bf16, tag=f"v{j}", name=f"v{j}{R}") for j in range(NT)]
        zT_sb = [sb.tile([128, S], bf16, tag=f"zT{p}", name=f"zT{p}{R}") for p in range(NPAIR)]

        # ---- v projection (natural layout), bias add + ones col ----
        def vproj(j):
            pv = psum.tile([128, 512], f32, name=f"pv{j}{R}", tag="pj", bufs=2)
            for c in range(NCH):
                nc.tensor.matmul(
                    pv[:, 0:384],
                    lhsT=xT_sb[:, c, j * 128:(j + 1) * 128],
                    rhs=wv_sb[:, c, :],
                    start=(c == 0),
                    stop=(c == NCH - 1),
                )
            nc.vector.memset(v_sb[j][:, :, 64:65], 1.0)
            nc.vector.tensor_add(
                v_sb[j][:, :, 0:64],
                pv[:, 0:384].rearrange("k (h e) -> k h e", h=NHL),
                bvb_sb[:].rearrange("k (h e) -> k h e", h=NHL),
            )

        def qk_nq(p, nq):
            # attention(p, cq) only reads q/k chunks nq <= cq, so these units
            # interleave finely with the attention calls.
            for half, (w_sb, dst) in enumerate(((wq_sb, qT_sb[p]), (wk_sb, kT_sb[p]))):
                ps = psum.tile([128, 512], f32, name=f"pr{p}_{half}_{nq}{R}", tag="pj", bufs=2)
                for c in range(NCH):
                    nc.tensor.matmul(
                        ps[:],
                        lhsT=w_sb[:, p, c, :],
                        rhs=xT_sb[:, c, nq * 512:(nq + 1) * 512],
                        start=(c == 0),
                        stop=(c == NCH - 1),
                    )
                nc.vector.tensor_scalar_add(
                    dst[:, nq * 512:(nq + 1) * 512],
                    ps[:],
                    bqk_sb[:, 2 * p + half:2 * p + half + 1],
                )

        def attention(p, cq):
            jmax = 4 * cq + 3
            # j=0 first (dep chain for z starts there), then the diagonal
            # blocks (whose mask dep chain is longest), then the rest
            jorder = [0] + list(range(max(4 * cq, 1), jmax + 1)) + list(range(1, 4 * cq))
            pts = {}
            for j in jorder:
                sqs = max(512 * cq, 128 * j)
                n = 512 - (sqs - 512 * cq)
                # one [128, 1024] psum tile: cols 0:512 head A, 512:1024 head B
                ps = psum.tile([128, 2, 512], f32, name=f"st{p}_{cq}_{j}{R}", tag="st", bufs=2)
                for h in range(2):
                    nc.tensor.matmul(
                        ps[:, h, :n],
                        lhsT=kT_sb[p][64 * h:64 * h + 64, j * 128:(j + 1) * 128],
                        rhs=qT_sb[p][64 * h:64 * h + 64, sqs:sqs + n],
                    )
                pt = sb.tile([128, 2, 512], bf16, name=f"pt{p}_{cq}_{j}{R}", tag="pt", bufs=16)
                pts[j] = pt
                nc.scalar.activation(pt[:, :, :n], ps[:, :, :n], AF.Exp, scale=0.125)
                if j >= 4 * cq:
                    # diagonal block: causal mask (keep sk <= sq)
                    nc.gpsimd.tensor_mul(pt[:, 0, 0:128], pt[:, 0, 0:128], mask_sb[:])
                    nc.gpsimd.tensor_mul(pt[:, 1, 0:128], pt[:, 1, 0:128], mask_sb[:])

            # ---- z in [he, sq] orientation: wide-N matmuls keep the PE
            # sequencer instruction count low (the wall tracks ~127ns per PE
            # instruction); the ones row of v puts the softmax denominator in
            # row 64 of each pz.
            pz = [
                psum.tile([65, 512], f32, name=f"pz{p}_{cq}_{h}{R}", tag=f"pz{h}", bufs=1)
                for h in range(2)
            ]
            for ji, j in enumerate(jorder):
                sqs = max(512 * cq, 128 * j)
                off = sqs - 512 * cq
                n = 512 - off
                for h in range(2):
                    nc.tensor.matmul(
                        pz[h][:, off:off + n],
                        lhsT=v_sb[j][:, p * 2 + h, :],
                        rhs=pts[j][:, h, :n],
                        start=(ji == 0),
                        stop=(ji == jmax),
                    )
            for h in range(2):
                # normalize: route the [1, 512] denominator row through the PE
                # (4 tiny transposes) into per-partition [128, 4] form, where
                # DVE reciprocal runs at full rate (free size 4, not 512 —
                # reciprocal is an iterative divide, free-size-bound and ~5x
                # slow on HW), then transpose back, broadcast on Pool, and
                # multiply the unnormalized z rows on DVE.
                dencp = sb.tile([1, 512], f32, name=f"den{p}_{cq}_{h}{R}", tag="den", bufs=2)
                nc.vector.tensor_copy(dencp[:], pz[h][64:65, :])
                tpd = psum.tile([128, 512], f32, name=f"tpd{p}_{cq}_{h}{R}", tag="pj", bufs=2)
                tpdb = tpd[:].bitcast(bf16)
                for t4 in range(4):
                    nc.tensor.matmul(
                        tpd[:, t4:t4 + 1],
                        lhsT=dencp[0:1, 128 * t4:128 * (t4 + 1)],
                        rhs=one_sb[:],
                        is_transpose=True,
                    )
                recp = sb.tile([128, 4], bf16, name=f"recp{p}_{cq}_{h}{R}", tag="recp", bufs=2)
                with nc.allow_low_precision(reason="softmax denom recip in bf16"):
                    nc.vector.reciprocal(recp[:], tpd[:, 0:4])
                for t4 in range(4):
                    nc.tensor.matmul(
                        tpdb[0:1, 256 + 128 * t4:256 + 128 * (t4 + 1)],
                        lhsT=recp[:, t4:t4 + 1],
                        rhs=eye_sb[:],
                        is_transpose=True,
                    )
                rec = sb.tile([1, 512], bf16, name=f"rec{p}_{cq}_{h}{R}", tag="rec", bufs=2)
                nc.vector.tensor_copy(rec[:], tpdb[0:1, 256:768])
                recb = sb.tile([64, 512], bf16, name=f"recb{p}_{cq}_{h}{R}", tag="recb", bufs=2)
                nc.gpsimd.partition_broadcast(recb[:], rec[:])
                nc.vector.tensor_mul(
                    zT_sb[p][64 * h:64 * h + 64, 512 * cq:512 * (cq + 1)],
                    recb[:],
                    pz[h][0:64, :],
                )

        def outproj(t):
            osb = sb.tile([128, 768], bf16, name=f"osb{t}{R}", tag="osb", bufs=4)
            for dh in range(2):
                po = psum.tile([128, 384], f32, name=f"po{t}_{dh}{R}", tag="pj", bufs=2)
                for p in range(NPAIR):
                    nc.tensor.matmul(
                        po[:],
                        lhsT=zT_sb[p][:, t * 128:(t + 1) * 128],
                        rhs=wo_sb[:, p, dh * 384:(dh + 1) * 384],
                        start=(p == 0),
                        stop=(p == NPAIR - 1),
                    )
                nc.vector.tensor_copy(osb[:, dh * 384:(dh + 1) * 384], po[:])
            nc.sync.dma_start(out_d[t * 128:(t + 1) * 128, :], osb[:])

        # pair-major interleave: pair 0's attention starts right after its
        # nq=0 q/k chunk; later q/k chunk projections are spread between
        # attention calls so PE always has filler while ACT chews exps.
        # Out-projections interleave with pair 2 (they need all three pairs'
        # zT for a chunk, which completes when pair 2's chunk finishes).
        for j in range(4):
            vproj(j)
        qk_nq(0, 0)
        attention(0, 0)
        for j in range(4, 8):
            vproj(j)
        qk_nq(0, 1); qk_nq(1, 0)
        attention(0, 1)
        for j in range(8, 12):
            vproj(j)
        qk_nq(0, 2); qk_nq(1, 1)
        attention(0, 2)
        for j in range(12, NT):
            vproj(j)
        qk_nq(0, 3); qk_nq(1, 2)
        attention(0, 3)
        qk_nq(1, 3); qk_nq(2, 0)
        attention(1, 0)
        qk_nq(2, 1)
        attention(1, 1)
        qk_nq(2, 2)
        attention(1, 2)
        qk_nq(2, 3)
        attention(1, 3)
        for cq in range(NSQ):
            attention(2, cq)
            for t in range(4 * cq, 4 * cq + 4):
                outproj(t)


def make_in_maps(normalized_resid_pre, W_Q, b_Q, W_K, b_K, W_V, b_V, W_O, b_O):
    import ml_dtypes

    bf = ml_dtypes.bfloat16
    x = np.asarray(normalized_resid_pre, dtype=np.float32)
    W_Q = np.asarray(W_Q, np.float32)
    W_K = np.asarray(W_K, np.float32)
    W_V = np.asarray(W_V, np.float32)
    W_O = np.asarray(W_O, np.float32)
    b_Q = np.asarray(b_Q, np.float32)
    b_K = np.asarray(b_K, np.float32)
    b_V = np.asarray(b_V, np.float32)

    mask = (np.arange(128)[:, None] <= np.arange(128)[None, :]).astype(bf)

    xT_by_batch = []
    for b in range(B):
        xT = np.ascontiguousarray(x[b].T)            # [768, 2048]
        # piece (hf, c) = [128 d_model rows, 1024 seq cols], stored contiguously
        xT = xT.reshape(NCH, 128, 2, S // 2).transpose(2, 0, 1, 3).reshape(2 * NCH * 128, S // 2)
        xT_by_batch.append(np.ascontiguousarray(xT.astype(bf)))

    def wqk_arrange(W, h0):
        # [128(k), NPAIR, NCH, 128(m = hp*64+e)]
        w = W[h0:h0 + NHL].reshape(NPAIR, 2, NCH, 128, DH)   # p, hp, c, k, e
        w = w.transpose(0, 2, 3, 1, 4)                        # p, c, k, hp, e
        return np.ascontiguousarray(
            w.reshape(NPAIR, NCH, 128, 128).transpose(2, 0, 1, 3).reshape(128, NPAIR * NCH * 128).astype(bf))

    def wv_arrange(W, h0):
        # [128(k), NCH, 384(f = h*64+e)]
        w = W[h0:h0 + NHL].reshape(NHL, NCH, 128, DH)         # h, c, k, e
        w = w.transpose(2, 1, 0, 3)                           # k, c, h, e
        return np.ascontiguousarray(w.reshape(128, NCH * 384).astype(bf))

    def wo_arrange(W, h0):
        # [128(k = hp*64+e), NPAIR*768]
        w = W[h0:h0 + NHL].reshape(NPAIR, 2, DH, D)           # p, hp, e, d
        w = w.transpose(1, 2, 0, 3)                           # hp, e, p, d
        return np.ascontiguousarray(w.reshape(128, NPAIR * D).astype(bf))

    in_maps = []
    for core in range(8):
        b = core // 2
        h0 = (core % 2) * NHL
        bqk = np.zeros((128, 6), np.float32)
        for p in range(NPAIR):
            bqk[:, 2 * p] = b_Q[h0 + 2 * p:h0 + 2 * p + 2].reshape(128)
            bqk[:, 2 * p + 1] = b_K[h0 + 2 * p:h0 + 2 * p + 2].reshape(128)
        bvb = np.broadcast_to(b_V[h0:h0 + NHL].reshape(1, 384), (128, 384))
        in_maps.append({
            "xT": xT_by_batch[b],
            "wq": wqk_arrange(W_Q, h0),
            "wk": wqk_arrange(W_K, h0),
            "wv": wv_arrange(W_V, h0),
            "wo": wo_arrange(W_O, h0),
            "bqk": bqk,
            "bvb": np.ascontiguousarray(bvb),
            "maskT": mask,
            "eye": np.eye(128, dtype=bf),
        })
    return in_maps


def gather(results, b_O):
    out = np.zeros((B, S, D), np.float32)
    for b in range(B):
        out[b] = results[2 * b]["out"].astype(np.float32) + \
            results[2 * b + 1]["out"].astype(np.float32)
    out += np.asarray(b_O, np.float32)[None, None, :]
    return out


def kernel(normalized_resid_pre, W_Q, b_Q, W_K, b_K, W_V, b_V, W_O, b_O, _trace=False):
    from concourse.bass_utils import run_bass_kernel_spmd

    if "nc" not in _CACHE:
        _CACHE["nc"] = build_nc()
    nc = _CACHE["nc"]
    in_maps = make_in_maps(normalized_resid_pre, W_Q, b_Q, W_K, b_K, W_V, b_V, W_O, b_O)
    res = run_bass_kernel_spmd(nc, in_maps, list(range(8)), trace=_trace)
    _CACHE["last_result"] = res
    return gather(res.results, b_O)


# revision 26
# speedup vs baseline: 1.5427x; 1.5427x over previous
import sys

if "/opt/trn_rl_repo" not in sys.path:
    sys.path.insert(0, "/opt/trn_rl_repo")

import numpy as np

B, S, D, NH, DH = 4, 2048, 768, 12, 64
NHL = 6        # heads per core
NPAIR = 3      # head pairs per core
NCH = 6        # d_model chunks of 128
NT = 16        # seq tiles of 128
NSQ = 4        # sq chunks of 512

_CACHE = {}


def build_nc(body_reps=1):
    import concourse.tile as tile
    from concourse import mybir, bacc

    f32 = mybir.dt.float32
    bf16 = mybir.dt.bfloat16
    AF = mybir.ActivationFunctionType

    nc = bacc.Bacc("TRN2", target_bir_lowering=False, debug=False)

    xT_d = nc.dram_tensor("xT", [2 * NCH * 128, S // 2], bf16, kind="ExternalInput")
    wq_d = nc.dram_tensor("wq", [128, NPAIR * NCH * 128], bf16, kind="ExternalInput")
    wk_d = nc.dram_tensor("wk", [128, NPAIR * NCH * 128], bf16, kind="ExternalInput")
    wv_d = nc.dram_tensor("wv", [128, NCH * 384], bf16, kind="ExternalInput")
    wo_d = nc.dram_tensor("wo", [128, NPAIR * 768], bf16, kind="ExternalInput")
    bqk_d = nc.dram_tensor("bqk", [128, 6], f32, kind="ExternalInput")
    bvb_d = nc.dram_tensor("bvb", [128, 384], f32, kind="ExternalInput")
    mask_d = nc.dram_tensor("maskT", [128, 128], bf16, kind="ExternalInput")
    eye_d = nc.dram_tensor("eye", [128, 128], bf16, kind="ExternalInput")
    out_d = nc.dram_tensor("out", [S, D], bf16, kind="ExternalOutput")

    with tile.TileContext(nc) as tc:
        for rep in range(body_reps):
            _emit_body(nc, tc, tile, mybir, rep,
                       xT_d, wq_d, wk_d, wv_d, wo_d, bqk_d, bvb_d, mask_d, eye_d, out_d)

    nc.compile()
    return nc


def _emit_body(nc, tc, tile, mybir, rep,
               xT_d, wq_d, wk_d, wv_d, wo_d, bqk_d, bvb_d, mask_d, eye_d, out_d):
    f32 = mybir.dt.float32
    bf16 = mybir.dt.bfloat16
    AF = mybir.ActivationFunctionType
    R = f"r{rep}"

    with (
        tc.tile_pool(name=f"sb{rep}", bufs=1) as sb,
        tc.tile_pool(name=f"psum{rep}", bufs=1, space="PSUM") as psum,
    ):
        # ---- constants ----
        wo_sb = sb.tile([128, NPAIR, 768], bf16, tag="wo")
        bqk_sb = sb.tile([128, 6], f32, tag="bqk")
        bvb_sb = sb.tile([128, 384], f32, tag="bvb")
        mask_sb = sb.tile([128, 128], bf16, tag="mask")
        eye_sb = sb.tile([128, 128], bf16, tag="eye")
        zero_sb = sb.tile([128, 4 * 65], bf16, tag="zero")
        xT_sb = sb.tile([128, NCH, S], bf16, tag="xT")
        wq_sb = sb.tile([128, NPAIR, NCH, 128], bf16, tag="wq")
        wk_sb = sb.tile([128, NPAIR, NCH, 128], bf16, tag="wk")
        wv_sb = sb.tile([128, NCH, 384], bf16, tag="wv")

        # xT streams on the SP queue (hf-outer so the first 1024 seq positions
        # of every d_model chunk land first); weights go down the Activation
        # hwdge queue concurrently so v/qk projections can start early.
        for hf in range(2):
            for c in range(NCH):
                pc = (hf * NCH + c) * 128
                nc.sync.dma_start(
                    xT_sb[:, c, hf * (S // 2):(hf + 1) * (S // 2)],
                    xT_d[pc:pc + 128, :],
                )
        nc.scalar.dma_start(wv_sb[:], wv_d[:].rearrange("k (c f) -> k c f", c=NCH))
        nc.scalar.dma_start(bvb_sb[:], bvb_d[:])
        nc.scalar.dma_start(wq_sb[:], wq_d[:].rearrange("k (p c m) -> k p c m", p=NPAIR, c=NCH))
        nc.scalar.dma_start(wk_sb[:], wk_d[:].rearrange("k (p c m) -> k p c m", p=NPAIR, c=NCH))
        nc.scalar.dma_start(bqk_sb[:], bqk_d[:])
        nc.scalar.dma_start(mask_sb[:], mask_d[:])
        nc.scalar.dma_start(eye_sb[:], eye_d[:])
        nc.scalar.dma_start(wo_sb[:], wo_d[:].rearrange("k (p d) -> k p d", p=NPAIR))
        nc.vector.memset(zero_sb[:], 0.0)

        qT_sb = [sb.tile([128, S], bf16, tag=f"qT{p}", name=f"qT{p}{R}") for p in range(NPAIR)]
        kT_sb = [sb.tile([128, S], bf16, tag=f"kT{p}", name=f"kT{p}{R}") for p in range(NPAIR)]
        v_sb = [sb.tile([128, NHL, 65], bf16, tag=f"v{j}", name=f"v{j}{R}") for j in range(NT)]
        zT_sb = [sb.tile([128, S], bf16, tag=f"zT{p}", name=f"zT{p}{R}") for p in range(NPAIR)]

        # ---- v projection (natural layout), bias add + ones col ----
        def vproj(j):
            pv = psum.tile([128, 512], f32, name=f"pv{j}{R}", tag="pj", bufs=2)
            for c in range(NCH):
                nc.tensor.matmul(
                    pv[:, 0:384],
                    lhsT=xT_sb[:, c, j * 128:(j + 1) * 128],
                    rhs=wv_sb[:, c, :],
                    start=(c == 0),
                    stop=(c == NCH - 1),
                )
            nc.vector.memset(v_sb[j][:, :, 64:65], 1.0)
            nc.vector.tensor_add(
                v_sb[j][:, :, 0:64],
                pv[:, 0:384].rearrange("k (h e) -> k h e", h=NHL),
                bvb_sb[:].rearrange("k (h e) -> k h e", h=NHL),
            )

        def qk_nq(p, nq):
            # attention(p, cq) only reads q/k chunks nq <= cq, so these units
            # interleave finely with the attention calls.
            for half, (w_sb, dst) in enumerate(((wq_sb, qT_sb[p]), (wk_sb, kT_sb[p]))):
                ps = psum.tile([128, 512], f32, name=f"pr{p}_{half}_{nq}{R}", tag="pj", bufs=2)
                for c in range(NCH):
                    nc.tensor.matmul(
                        ps[:],
                        lhsT=w_sb[:, p, c, :],
                        rhs=xT_sb[:, c, nq * 512:(nq + 1) * 512],
                        start=(c == 0),
                        stop=(c == NCH - 1),
                    )
                nc.vector.tensor_scalar_add(
                    dst[:, nq * 512:(nq + 1) * 512],
                    ps[:],
                    bqk_sb[:, 2 * p + half:2 * p + half + 1],
                )

        def attention(p, cq):
            jmax = 4 * cq + 3
            # j=0 first (dep chain for z starts there), then the diagonal
            # blocks (whose mask dep chain is longest), then the rest
            jorder = [0] + list(range(max(4 * cq, 1), jmax + 1)) + list(range(1, 4 * cq))
            pts = {}
            for j in jorder:
                sqs = max(512 * cq, 128 * j)
                n = 512 - (sqs - 512 * cq)
                # one [128, 1024] psum tile: cols 0:512 head A, 512:1024 head B
                ps = psum.tile([128, 2, 512], f32, name=f"st{p}_{cq}_{j}{R}", tag="st", bufs=2)
                for h in range(2):
                    nc.tensor.matmul(
                        ps[:, h, :n],
                        lhsT=kT_sb[p][64 * h:64 * h + 64, j * 128:(j + 1) * 128],
                        rhs=qT_sb[p][64 * h:64 * h + 64, sqs:sqs + n],
                    )
                pt = sb.tile([128, 2, 512], bf16, name=f"pt{p}_{cq}_{j}{R}", tag="pt", bufs=16)
                pts[j] = pt
                nc.scalar.activation(pt[:, :, :n], ps[:, :, :n], AF.Exp, scale=0.125)
                if j >= 4 * cq:
                    # diagonal block: causal mask (keep sk <= sq)
                    nc.gpsimd.tensor_mul(pt[:, 0, 0:128], pt[:, 0, 0:128], mask_sb[:])
                    nc.gpsimd.tensor_mul(pt[:, 1, 0:128], pt[:, 1, 0:128], mask_sb[:])

            # ---- z in transposed orientation: pz[sq, 65] per (h, sq-tile),
            # all 4 sq-tiles of one head accumulate as ONE psum group in ONE
            # bank ([128, 4, 65]); a full-tile zero matmul opens the group
            # (start=True, WAW-orders everything after it) and another closes
            # it (stop=True, executes last by the same WAW argument).
            pzs = []
            for h in range(2):
                pz = psum.tile([128, 4, 65], f32, name=f"pz{p}_{cq}_{h}{R}", tag="pz", bufs=2)
                pzs.append(pz)
                nc.tensor.matmul(
                    pz[:].rearrange("k t e -> k (t e)"),
                    lhsT=eye_sb[:],
                    rhs=zero_sb[:, 0:260],
                    start=True, stop=False, skip_group_check=True,
                )
            for j in jorder:
                for tt in range(4):
                    tg = 4 * cq + tt
                    if j > tg:
                        continue
                    sqs = max(512 * cq, 128 * j)
                    off = 128 * tg - sqs
                    for h in range(2):
                        nc.tensor.matmul(
                            pzs[h][:, tt, :],
                            lhsT=pts[j][:, h, off:off + 128],
                            rhs=v_sb[j][:, p * 2 + h, :],
                            start=False, stop=False, skip_group_check=True,
                        )
            for h in range(2):
                nc.tensor.matmul(
                    pzs[h][:].rearrange("k t e -> k (t e)"),
                    lhsT=eye_sb[:],
                    rhs=zero_sb[:, 0:260],
                    start=False, stop=True, skip_group_check=True,
                )

            # ---- normalize: den sits at col 64 of each pz; per-partition
            # reciprocal then broadcast multiply (all per-sq lanes)
            tp = psum.tile([128, 512], f32, name=f"tp{p}_{cq}{R}", tag="pj", bufs=2)
            tpb = tp[:].bitcast(bf16)[:, 0:512].rearrange("k (t s) -> k t s", t=4)
            for h in range(2):
                rec = sb.tile([128, 4], f32, name=f"rec{p}_{cq}_{h}{R}", tag="rec", bufs=4)
                nc.vector.reciprocal(rec[:], pzs[h][:, :, 64])
                zn = sb.tile([128, 4, 64], bf16, name=f"zn{p}_{cq}_{h}{R}", tag="zn", bufs=4)
                nc.vector.tensor_mul(
                    zn[:],
                    pzs[h][:, :, 0:64],
                    rec[:].unsqueeze(2).broadcast_to([128, 4, 64]),
                )
                # transpose each 128x64 sq-tile back to [he, sq] layout
                for tt in range(4):
                    nc.tensor.matmul(
                        tpb[64 * h:64 * h + 64, tt, :],
                        lhsT=zn[:, tt, :],
                        rhs=eye_sb[:],
                        is_transpose=True,
                    )
            nc.vector.tensor_copy(
                zT_sb[p][:, 512 * cq:512 * (cq + 1)],
                tpb[:].rearrange("k t s -> k (t s)"),
            )

        def outproj(t):
            osb = sb.tile([128, 768], bf16, name=f"osb{t}{R}", tag="osb", bufs=4)
            for dh in range(2):
                po = psum.tile([128, 384], f32, name=f"po{t}_{dh}{R}", tag="pj", bufs=2)
                for p in range(NPAIR):
                    nc.tensor.matmul(
                        po[:],
                        lhsT=zT_sb[p][:, t * 128:(t + 1) * 128],
                        rhs=wo_sb[:, p, dh * 384:(dh + 1) * 384],
                        start=(p == 0),
                        stop=(p == NPAIR - 1),
                    )
                nc.vector.tensor_copy(osb[:, dh * 384:(dh + 1) * 384], po[:])
            nc.sync.dma_start(out_d[t * 128:(t + 1) * 128, :], osb[:])

        # pair-major interleave: pair 0's attention starts right after its
        # nq=0 q/k chunk; later q/k chunk projections are spread between
        # attention calls so PE always has filler while ACT chews exps.
        # Out-projections interleave with pair 2 (they need all three pairs'
        # zT for a chunk, which completes when pair 2's chunk finishes).
        for j in range(4):
            vproj(j)
        qk_nq(0, 0)
        attention(0, 0)
        for j in range(4, 8):
            vproj(j)
        qk_nq(0, 1); qk_nq(1, 0)
        attention(0, 1)
        for j in range(8, 12):
            vproj(j)
        qk_nq(0, 2); qk_nq(1, 1)
        attention(0, 2)
        for j in range(12, NT):
            vproj(j)
        qk_nq(0, 3); qk_nq(1, 2)
        attention(0, 3)
        qk_nq(1, 3); qk_nq(2, 0)
        attention(1, 0)
        qk_nq(2, 1)
        attention(1, 1)
        qk_nq(2, 2)
        attention(1, 2)
        qk_nq(2, 3)
        attention(1, 3)
        for cq in range(NSQ):
            attention(2, cq)
            for t in range(4 * cq, 4 * cq + 4):
                outproj(t)


def make_in_maps(normalized_resid_pre, W_Q, b_Q, W_K, b_K, W_V, b_V, W_O, b_O):
    import ml_dtypes

    bf = ml_dtypes.bfloat16
    x = np.asarray(normalized_resid_pre, dtype=np.float32)
    W_Q = np.asarray(W_Q, np.float32)
    W_K = np.asarray(W_K, np.float32)
    W_V = np.asarray(W_V, np.float32)
    W_O = np.asarray(W_O, np.float32)
    b_Q = np.asarray(b_Q, np.float32)
    b_K = np.asarray(b_K, np.float32)
    b_V = np.asarray(b_V, np.float32)

    mask = (np.arange(128)[:, None] <= np.arange(128)[None, :]).astype(bf)

    xT_by_batch = []
    for b in range(B):
        xT = np.ascontiguousarray(x[b].T)            # [768, 2048]
        # piece (hf, c) = [128 d_model rows, 1024 seq cols], stored contiguously
        xT = xT.reshape(NCH, 128, 2, S // 2).transpose(2, 0, 1, 3).reshape(2 * NCH * 128, S // 2)
        xT_by_batch.append(np.ascontiguousarray(xT.astype(bf)))

    def wqk_arrange(W, h0):
        # [128(k), NPAIR, NCH, 128(m = hp*64+e)]
        w = W[h0:h0 + NHL].reshape(NPAIR, 2, NCH, 128, DH)   # p, hp, c, k, e
        w = w.transpose(0, 2, 3, 1, 4)                        # p, c, k, hp, e
        return np.ascontiguousarray(
            w.reshape(NPAIR, NCH, 128, 128).transpose(2, 0, 1, 3).reshape(128, NPAIR * NCH * 128).astype(bf))

    def wv_arrange(W, h0):
        # [128(k), NCH, 384(f = h*64+e)]
        w = W[h0:h0 + NHL].reshape(NHL, NCH, 128, DH)         # h, c, k, e
        w = w.transpose(2, 1, 0, 3)                           # k, c, h, e
        return np.ascontiguousarray(w.reshape(128, NCH * 384).astype(bf))

    def wo_arrange(W, h0):
        # [128(k = hp*64+e), NPAIR*768]
        w = W[h0:h0 + NHL].reshape(NPAIR, 2, DH, D)           # p, hp, e, d
        w = w.transpose(1, 2, 0, 3)                           # hp, e, p, d
        return np.ascontiguousarray(w.reshape(128, NPAIR * D).astype(bf))

    in_maps = []
    for core in range(8):
        b = core // 2
        h0 = (core % 2) * NHL
        bqk = np.zeros((128, 6), np.float32)
        for p in range(NPAIR):
            bqk[:, 2 * p] = b_Q[h0 + 2 * p:h0 + 2 * p + 2].reshape(128)
            bqk[:, 2 * p + 1] = b_K[h0 + 2 * p:h0 + 2 * p + 2].reshape(128)
        bvb = np.broadcast_to(b_V[h0:h0 + NHL].reshape(1, 384), (128, 384))
        in_maps.append({
            "xT": xT_by_batch[b],
            "wq": wqk_arrange(W_Q, h0),
            "wk": wqk_arrange(W_K, h0),
            "wv": wv_arrange(W_V, h0),
            "wo": wo_arrange(W_O, h0),
            "bqk": bqk,
            "bvb": np.ascontiguousarray(bvb),
            "maskT": mask,
            "eye": np.eye(128, dtype=bf),
        })
    return in_maps


def gather(results, b_O):
    out = np.zeros((B, S, D), np.float32)
    for b in range(B):
        out[b] = results[2 * b]["out"].astype(np.float32) + \
            results[2 * b + 1]["out"].astype(np.float32)
    out += np.asarray(b_O, np.float32)[None, None, :]
    return out


def kernel(normalized_resid_pre, W_Q, b_Q, W_K, b_K, W_V, b_V, W_O, b_O, _trace=False):
    from concourse.bass_utils import run_bass_kernel_spmd

    if "nc" not in _CACHE:
        _CACHE["nc"] = build_nc()
    nc = _CACHE["nc"]
    in_maps = make_in_maps(normalized_resid_pre, W_Q, b_Q, W_K, b_K, W_V, b_V, W_O, b_O)
    res = run_bass_kernel_spmd(nc, in_maps, list(range(8)), trace=_trace)
    _CACHE["last_result"] = res
    return gather(res.results, b_O)
